# revision 18
# baseline (speedup 1.0000x reference)
"""LIF neuron step on 8 Trainium2 NeuronCores.

Math (reference):
    I_raw   = g @ w                       # [N] vec-mat product, w is [N, N]
    I       = sigmoid(12/N * I_raw) + 0.9 * x_in
    v_next  = v + (E_L - v + I * (30 - E_L)) / tau_m
    out     = sigmoid(v_next - 30)

Everything after the matvec is affine in I_sig = sigmoid(12/N * I_raw):
    out = sigmoid(B * I_sig + D)
    B   = (30 - E_L) / tau_m
    D   = v + (E_L - v)/tau_m - 30 + 0.9 * x_in * B
B and D are tiny per-neuron vectors, computed on the host.

Sharding: w is split column-wise (output-neuron dim) into 8 shards of
[8192, 1024]; g is replicated. Each core computes its 1024 outputs fully
locally; host concatenates. The kernel is memory-bound on streaming the
w shard; w/g are cast to fp16 on the host (absmax-relative output error
~1e-5) which halves HBM traffic. PE does the matvec with w-tiles as the
stationary operand so the per-core result lands as a [128, 8] tile and
the elementwise tail uses all 128 lanes.
"""

from contextlib import ExitStack

import numpy as np

import concourse.bass as bass
import concourse.bacc as bacc
import concourse.mybir as mybir
import concourse.tile as tile
from concourse.bass_utils import run_bass_kernel_spmd

N = 8192          # neurons
NCORES = 8
COLS = N // NCORES  # 1024 output neurons per core
P = 128           # partitions
KT = N // P       # 64 contraction tiles
CHUNK_SIZES = [4, 12, 16, 16, 16]  # k-tiles per DMA chunk (sums to KT)
JT = COLS // P    # 8 output tiles per core
SPIKE = 30.0

TRACE = False          # set True to capture NTFF profile
LAST_RESULT = None     # BassKernelResults of the most recent run

_NC = None


def _build():
    nc = bacc.Bacc("TRN2", target_bir_lowering=False, debug=False,
                   num_devices=NCORES)
    wt = nc.dram_tensor("wt", [N, COLS], mybir.dt.float16,
                        kind="ExternalInput").ap()
    gt = nc.dram_tensor("gt", [P, KT], mybir.dt.float16,
                        kind="ExternalInput").ap()
    bd = nc.dram_tensor("bd", [P, 3 * JT], mybir.dt.float32,
                        kind="ExternalInput").ap()
    out = nc.dram_tensor("out", [P, JT], mybir.dt.float32,
                         kind="ExternalOutput").ap()

    # partition p, free (t, c)  <-  w row t*P + p, col c
    wtk = wt.rearrange("(t p) c -> p t c", p=P)

    with tile.TileContext(nc) as tc, ExitStack() as ctx:
        wpool = ctx.enter_context(tc.tile_pool(name="w", bufs=1))
        spool = ctx.enter_context(tc.tile_pool(name="s", bufs=1))
        ppool = ctx.enter_context(tc.tile_pool(name="p", bufs=1, space="PSUM"))

        gsb = spool.tile([P, KT], mybir.dt.float16)
        nc.sync.dma_start(gsb[:], gt[:])
        bdsb = spool.tile([P, 3 * JT], mybir.dt.float32)
        nc.sync.dma_start(bdsb[:], bd[:])
        # Pre-touch bdsb on ACT so later activations need no new DMA wait
        # (per-instruction sync-wait slots are scarce in the NEFF encoding).
        pre = spool.tile([P, 1], mybir.dt.float32)
        nc.scalar.copy(pre[:], bdsb[:, 0:1])

        acc = ppool.tile([P, JT], mybir.dt.float32)
        # Unequal chunks: small first chunk so PE starts early; 5 chunk
        # DMAs + 3 small DMAs = 8 HWDGE lanes, each used exactly once.
        k0 = 0
        for ct in CHUNK_SIZES:
            wsb = wpool.tile([P, ct * COLS], mybir.dt.float16, tag=f"w{k0}")
            nc.sync.dma_start(wsb[:].rearrange("p (t c) -> p t c", t=ct),
                              wtk[:, k0:k0 + ct, :])
            for t in range(ct):
                ki = k0 + t
                for jt in range(JT):
                    nc.tensor.matmul(
                        acc[:, jt:jt + 1],
                        wsb[:, t * COLS + jt * P: t * COLS + (jt + 1) * P],
                        gsb[:, ki:ki + 1],
                        start=(ki == 0 and jt == 0),
                        stop=(ki == KT - 1 and jt == JT - 1),
                    )
            k0 += ct

        # Tail entirely on ACT: out = sigmoid(B * sigmoid(acc*12/N) + D),
        # with B/D applied per j-tile as per-partition scale/bias APs.
        isig = spool.tile([P, JT], mybir.dt.float32)
        res = spool.tile([P, JT], mybir.dt.float32)
        for jt in range(JT):
            nc.scalar.activation(isig[:, jt:jt + 1], acc[:, jt:jt + 1],
                                 mybir.ActivationFunctionType.Sigmoid,
                                 scale=12.0 / N,
                                 bias=bdsb[:, 2 * JT + jt:2 * JT + jt + 1])
        for jt in range(JT):
            nc.scalar.activation(res[:, jt:jt + 1], isig[:, jt:jt + 1],
                                 mybir.ActivationFunctionType.Sigmoid,
                                 scale=bdsb[:, jt:jt + 1],
                                 bias=bdsb[:, JT + jt:JT + jt + 1])
        nc.sync.dma_start(out[:], res[:])
    nc.compile()
    return nc


def make_in_maps(x_in, v, g, w, E_L, tau_m):
    w16 = np.asarray(w).astype(np.float16)
    g16t = np.ascontiguousarray(
        np.asarray(g).astype(np.float16).reshape(KT, P).T)

    E = np.asarray(E_L, dtype=np.float64)
    TM = np.asarray(tau_m, dtype=np.float64)
    V = np.asarray(v, dtype=np.float64)
    X = np.asarray(x_in, dtype=np.float64)
    B = (SPIKE - E) / TM
    D = V + (E - V) / TM - SPIKE + 0.9 * X * B

    in_maps = []
    for c in range(NCORES):
        sl = slice(c * COLS, (c + 1) * COLS)
        bdc = np.concatenate(
            [B[sl].astype(np.float32).reshape(JT, P).T,
             D[sl].astype(np.float32).reshape(JT, P).T,
             np.zeros((P, JT), dtype=np.float32)], axis=1)
        in_maps.append({
            "wt": np.ascontiguousarray(w16[:, sl]),
            "gt": g16t,
            "bd": np.ascontiguousarray(bdc),
        })
    return in_maps


def kernel(x_in, v, g, w, E_L, tau_m, tau_g=None, **_unused):
    global _NC, LAST_RESULT
    if _NC is None:
        _NC = _build()
    in_maps = make_in_maps(x_in, v, g, w, E_L, tau_m)
    LAST_RESULT = run_bass_kernel_spmd(_NC, in_maps, list(range(NCORES)),
                                       trace=TRACE)
    out = np.empty(N, dtype=np.float32)
    for c in range(NCORES):
        out[c * COLS:(c + 1) * COLS] = \
            LAST_RESULT.results[c]["out"].T.reshape(COLS)
    return out
